# revision 13
# baseline (speedup 1.0000x reference)
"""Trainium2 Bass kernel for a 4-layer LSTM single decode step (T=1).

B=8192, H=1024, L=4, data-parallel over 8 NeuronCores (1024 batch rows
per core), LSTM/Dense weights replicated.

Key structure (per core):
  - Activations live transposed in SBUF: hT/cT are [H, B_shard] so the
    gate GEMM zT[n, m] = sum_k W[k, n] * hT[k, m] uses the weight tile
    (natural [K, N] layout) as the stationary operand and hT as the
    moving operand. No on-chip transposes anywhere.
  - With T=1 the per-layer input x_t and hidden h are the same tensor for
    layers 1..3, so z = h @ (Wx + Wh) + b; the weight sum is folded on the
    host. Layer 0's x contribution (x outer Wx0) is a DVE
    scalar_tensor_tensor accumulate into PSUM, keeping it off the PE.
  - Matmuls run in fp16 (fp32 PSUM accumulation); gate math in fp32.
  - DMA order puts the first weight slabs + h chunk 0 ahead of the bulk
    c/h loads so the PE starts as early as possible.
"""

import numpy as np

import concourse.bass as bass
import concourse.mybir as mybir
import bass_rust
from concourse.tile import TileContext
from concourse.bass_utils import run_bass_kernel_spmd

NPF16 = np.float16
F32 = mybir.dt.float32
F16 = mybir.dt.float16

B, H, L = 8192, 1024, 4
NCORES = 8
BS = B // NCORES          # batch shard per core (1024)
KT = H // 128             # k tiles (8)
JT = H // 128             # h-tile count (8)
FCH = 512                 # batch chunk per matmul (PSUM bank limit, fp32)
NF = BS // FCH            # chunks (2)

_ACT = mybir.ActivationFunctionType
_ALU = mybir.AluOpType


def _patch_bass():
    """Work around two walrus limitations in this container:
    1. the raw-ISA EVENT_SEMAPHORE_RANGE_CLEAR Tile emits at exit doesn't
       codegen -> skip the clears, keep the sem accounting;
    2. instructions with more than 1 sync wait fail setupSyncWait ->
       split extras onto EventSemaphore carriers (see _split_multiwaits).
    """
    def _cafs(self, sems):
        if not sems:
            return
        sem_nums = [s.num if hasattr(s, "num") else s for s in sems]
        self._state.prepend_free_semaphores(sem_nums)
        for ps in self._tile_sem_poison_stack:
            ps.update(sem_nums)

    bass.Bass.clear_and_free_semaphores = _cafs


def _split_multiwaits(nc):
    for blk in nc.m.functions[0].blocks:
        insts = blk.instructions
        i = 0
        while i < len(insts):
            inst = insts[i]
            si = inst.sync_info
            cap = 1
            if si is not None and len(si.on_wait) > cap:
                waits = list(si.on_wait)
                si.on_wait = waits[:cap]
                extra = waits[cap:]
                k = 0
                while extra:
                    chunk, extra = extra[:1], extra[1:]
                    carrier = mybir.InstEventSemaphore(
                        name=f"{inst.name}-ws{k}", engine=inst.engine
                    )
                    carrier.sync_info = bass_rust.SyncInfo(
                        on_wait=chunk, on_update=[]
                    )
                    insts.insert(i, carrier)
                    i += 1
                    k += 1
            i += 1


def build_module(split=True):
    _patch_bass()
    nc = bass.Bass("TRN2", target_bir_lowering=False, debug=False)

    cT0 = nc.dram_tensor("cT0", [H, BS], F32, kind="ExternalInput").ap()
    hT0 = nc.dram_tensor("hT0", [H, BS], F16, kind="ExternalInput").ap()
    xB = nc.dram_tensor("xB", [128, BS], F16, kind="ExternalInput").ap()
    w0 = nc.dram_tensor("w0", [JT, 128, 4 * H], F16, kind="ExternalInput").ap()
    wl = nc.dram_tensor("wl", [L - 1, JT, 128, 4 * H], F16, kind="ExternalInput").ap()
    wx0 = nc.dram_tensor("wx0", [128, 32], F32, kind="ExternalInput").ap()
    bias = nc.dram_tensor("bias", [128, L * 32], F32, kind="ExternalInput").ap()
    wd = nc.dram_tensor("wd", [128, KT], F16, kind="ExternalInput").ap()
    bd = nc.dram_tensor("bd", [1, 1], F32, kind="ExternalInput").ap()

    cT_out = nc.dram_tensor("cT_out", [H, BS], F32, kind="ExternalOutput").ap()
    hT_out = nc.dram_tensor("hT_out", [H, BS], F32, kind="ExternalOutput").ap()
    xp_out = nc.dram_tensor("xp_out", [1, BS], F32, kind="ExternalOutput").ap()

    with TileContext(nc) as tc:
        with (
            tc.tile_pool(name="persist", bufs=1) as persist,
            tc.tile_pool(name="wpool", bufs=4) as wpool,
            tc.tile_pool(name="psum", bufs=6, space="PSUM") as psum,
            tc.tile_pool(name="dpsum", bufs=2, space="PSUM") as dpsum,
            tc.tile_pool(name="gact", bufs=2) as gact,
            tc.tile_pool(name="gtmp", bufs=2) as gtmp,
            tc.tile_pool(name="hout", bufs=3) as hout,
        ):
            # persistent state
            c_t = [persist.tile([128, BS], F32, tag=f"c{j}", name=f"c{j}") for j in range(JT)]
            h_a = [persist.tile([128, BS], F16, tag=f"ha{j}", name=f"ha{j}") for j in range(JT)]
            h_b = [persist.tile([128, BS], F16, tag=f"hb{j}", name=f"hb{j}") for j in range(JT)]
            bias_sb = persist.tile([128, L * 32], F32, tag="bias")
            x_sb = persist.tile([128, BS], F16, tag="x")
            wx0_sb = persist.tile([128, 32], F32, tag="wx0")
            wd_sb = persist.tile([128, KT], F16, tag="wd")
            bd_sb = persist.tile([1, 1], F32, tag="bd")

            w_tiles = {}

            def load_w(l, j):
                t = wpool.tile([128, 4 * H], F16, tag="w", name=f"w{l}_{j}")
                src = w0[j] if l == 0 else wl[l - 1][j]
                # split across DMA queues: one queue moves only ~50GB/s
                for q in range(4):
                    qs = slice(q * H, (q + 1) * H)
                    nc.sync.dma_start(out=t[:, qs], in_=src[:, qs])
                return t

            def load_c(j):
                for q in range(NF):
                    qs = slice(q * FCH, (q + 1) * FCH)
                    nc.sync.dma_start(out=c_t[j][:, qs],
                                      in_=cT0[j * 128:(j + 1) * 128, qs])

            # PE-critical loads first: weight slab j=0 (HWDGE queues) +
            # h chunk 0 on the SWDGE queues so the burst uses all 16.
            w_tiles[(0, 0)] = load_w(0, 0)
            for j in range(JT):
                nc.gpsimd.dma_start(out=h_a[j][:, 0:FCH],
                                    in_=hT0[j * 128:(j + 1) * 128, 0:FCH])
            nc.sync.dma_start(out=bias_sb, in_=bias[:, :])
            nc.sync.dma_start(out=wx0_sb, in_=wx0[:, :])
            nc.sync.dma_start(out=wd_sb, in_=wd[:, :])
            nc.sync.dma_start(out=bd_sb, in_=bd[:, :])
            load_c(0)
            nc.sync.dma_start(out=x_sb, in_=xB[:, :])
            for j in range(JT):
                nc.sync.dma_start(out=h_a[j][:, FCH:BS],
                                  in_=hT0[j * 128:(j + 1) * 128, FCH:BS])
            w_tiles[(0, 1)] = load_w(0, 1)
            load_c(1)
            w_tiles[(0, 2)] = load_w(0, 2)
            w_tiles[(0, 3)] = load_w(0, 3)
            for j in range(2, JT):
                load_c(j)

            for l in range(L):
                h_in = h_a if l % 2 == 0 else h_b
                h_out = h_b if l % 2 == 0 else h_a
                for j in range(JT):
                    w_sb = w_tiles.pop((l, j), None)
                    if w_sb is None:
                        w_sb = load_w(l, j)
                    for f in range(NF):
                        fsl = slice(f * FCH, (f + 1) * FCH)
                        ps = []
                        for g in range(4):
                            p = psum.tile([128, FCH], F32, tag="ps", name="ps")
                            ps.append(p)
                            for k in range(KT):
                                nc.tensor.matmul(
                                    p,
                                    lhsT=w_sb[:, (k * 4 + g) * 128:(k * 4 + g + 1) * 128],
                                    rhs=h_in[k][:, fsl],
                                    start=(k == 0),
                                    stop=(k == KT - 1),
                                )
                            if l == 0:
                                # += Wx0[n] * x[m]  (rank-1 x-term on DVE)
                                nc.vector.scalar_tensor_tensor(
                                    out=p,
                                    in0=x_sb[:, fsl],
                                    scalar=wx0_sb[:, j * 4 + g:j * 4 + g + 1],
                                    in1=p,
                                    op0=_ALU.mult,
                                    op1=_ALU.add,
                                )
                        bcol = lambda g: bias_sb[:, l * 32 + j * 4 + g:l * 32 + j * 4 + g + 1]
                        i_t = gact.tile([128, FCH], F32, tag="i")
                        nc.scalar.activation(i_t, ps[0], _ACT.Sigmoid, bias=bcol(0))
                        f_t = gact.tile([128, FCH], F32, tag="f")
                        nc.scalar.activation(f_t, ps[1], _ACT.Sigmoid, bias=bcol(1))
                        g_t = gact.tile([128, FCH], F32, tag="g")
                        nc.scalar.activation(g_t, ps[2], _ACT.Tanh, bias=bcol(2))
                        o_t = gact.tile([128, FCH], F32, tag="o")
                        nc.scalar.activation(o_t, ps[3], _ACT.Sigmoid, bias=bcol(3))

                        csl = c_t[j][:, fsl]
                        t1 = gtmp.tile([128, FCH], F32, tag="t1")
                        nc.vector.tensor_mul(t1, f_t, csl)
                        t2 = gtmp.tile([128, FCH], F32, tag="t2")
                        nc.vector.tensor_mul(t2, i_t, g_t)
                        nc.vector.tensor_add(csl, t1, t2)
                        th = gtmp.tile([128, FCH], F32, tag="th")
                        nc.scalar.activation(th, csl, _ACT.Tanh)
                        if l < L - 1:
                            nc.vector.tensor_mul(h_out[j][:, fsl], o_t, th)
                        else:
                            hf = hout.tile([128, FCH], F32, tag="hf")
                            nc.vector.tensor_mul(hf, o_t, th)
                            nc.vector.tensor_copy(h_out[j][:, fsl], hf)
                            nc.sync.dma_start(
                                out=hT_out[j * 128:(j + 1) * 128, fsl], in_=hf
                            )
                            nc.sync.dma_start(
                                out=cT_out[j * 128:(j + 1) * 128, fsl], in_=csl
                            )

            # dense head: x_pred = h4 @ Wd + bd  ->  [1, BS]
            h_fin = h_a if L % 2 == 0 else h_b
            for f in range(NF):
                fsl = slice(f * FCH, (f + 1) * FCH)
                dp = dpsum.tile([1, FCH], F32, tag="dp")
                for k in range(KT):
                    nc.tensor.matmul(
                        dp,
                        lhsT=wd_sb[:, k:k + 1],
                        rhs=h_fin[k][:, fsl],
                        start=(k == 0),
                        stop=(k == KT - 1),
                    )
                xp_sb = hout.tile([1, FCH], F32, tag="xp")
                nc.scalar.activation(xp_sb, dp, _ACT.Identity, bias=bd_sb[0:1, 0:1])
                nc.sync.dma_start(out=xp_out[0:1, fsl], in_=xp_sb)

    if split:
        _split_multiwaits(nc)
    return nc


def preprocess(x_todec, c0, h0, Wx0, Wh0, b0, Wx, Wh, b, Wd, bd):
    """Host-side packing into the per-core DRAM layouts."""
    def pack_w(W):
        # W: [H, 4H] -> [j, p, (k g n')] with value W[k*128+p, g*1024+j*128+n']
        Wr = W.reshape(KT, 128, 4, JT, 128)           # [k, p, g, j, n']
        return np.ascontiguousarray(
            Wr.transpose(3, 1, 0, 2, 4).reshape(JT, 128, 4 * H)
        ).astype(NPF16)

    w0_p = pack_w(Wh0)
    wl_p = np.stack([pack_w(Wx[i] + Wh[i]) for i in range(L - 1)])

    def pack_b(bv):
        # [4H] -> [p, j*4+g] with value bv[g*1024 + j*128 + p]
        return bv.reshape(4, JT, 128).transpose(2, 1, 0).reshape(128, 32)

    bias_p = np.ascontiguousarray(
        np.concatenate([pack_b(b0)] + [pack_b(b[i]) for i in range(L - 1)], axis=1)
    ).astype(np.float32)

    wx0_p = np.ascontiguousarray(pack_b(Wx0[0])).astype(np.float32)
    wd_p = np.ascontiguousarray(Wd.reshape(KT, 128).T).astype(NPF16)
    bd_p = bd.reshape(1, 1).astype(np.float32)

    in_maps = []
    for i in range(NCORES):
        sl = slice(i * BS, (i + 1) * BS)
        xb = np.broadcast_to(
            x_todec[sl].reshape(1, BS).astype(NPF16), (128, BS)
        ).copy()
        in_maps.append({
            "cT0": np.ascontiguousarray(c0[sl].T).astype(np.float32),
            "hT0": np.ascontiguousarray(h0[sl].T).astype(NPF16),
            "xB": xb,
            "w0": w0_p,
            "wl": wl_p,
            "wx0": wx0_p,
            "bias": bias_p,
            "wd": wd_p,
            "bd": bd_p,
        })
    return in_maps


def postprocess(results):
    c = np.empty((B, H), np.float32)
    h = np.empty((B, H), np.float32)
    xp = np.empty((B, 1), np.float32)
    for i in range(NCORES):
        sl = slice(i * BS, (i + 1) * BS)
        c[sl] = results[i]["cT_out"].T
        h[sl] = results[i]["hT_out"].T
        xp[sl, 0] = results[i]["xp_out"][0]
    return c, h, xp


_NC_CACHE = None


def kernel(x_todec, c0, h0, Wx0, Wh0, b0, Wx, Wh, b, Wd, bd, training=0, **_):
    global _NC_CACHE
    args = [np.asarray(a, np.float32) for a in
            (x_todec, c0, h0, Wx0, Wh0, b0, Wx, Wh, b, Wd, bd)]
    in_maps = preprocess(*args)
    if _NC_CACHE is None:
        _NC_CACHE = build_module()
    res = run_bass_kernel_spmd(_NC_CACHE, in_maps, list(range(NCORES)))
    return postprocess(res.results)


# revision 14
# speedup vs baseline: 1.0041x; 1.0041x over previous
"""Trainium2 Bass kernel for a 4-layer LSTM single decode step (T=1).

B=8192, H=1024, L=4, data-parallel over 8 NeuronCores (1024 batch rows
per core), LSTM/Dense weights replicated.

Key structure (per core):
  - Activations live transposed in SBUF: hT/cT are [H, B_shard] so the
    gate GEMM zT[n, m] = sum_k W[k, n] * hT[k, m] uses the weight tile
    (natural [K, N] layout) as the stationary operand and hT as the
    moving operand. No on-chip transposes anywhere.
  - With T=1 the per-layer input x_t and hidden h are the same tensor for
    layers 1..3, so z = h @ (Wx + Wh) + b; the weight sum is folded on the
    host. Layer 0's x contribution (x outer Wx0) is a DVE
    scalar_tensor_tensor accumulate into PSUM, keeping it off the PE.
  - Matmuls run in fp16 (fp32 PSUM accumulation); gate math in fp32.
  - DMA order puts the first weight slabs + h chunk 0 ahead of the bulk
    c/h loads so the PE starts as early as possible.
"""

import numpy as np

import concourse.bass as bass
import concourse.mybir as mybir
import bass_rust
from concourse.tile import TileContext
from concourse.bass_utils import run_bass_kernel_spmd

NPF16 = np.float16
F32 = mybir.dt.float32
F16 = mybir.dt.float16

B, H, L = 8192, 1024, 4
NCORES = 8
BS = B // NCORES          # batch shard per core (1024)
KT = H // 128             # k tiles (8)
JT = H // 128             # h-tile count (8)
FCH = 512                 # batch chunk per matmul (PSUM bank limit, fp32)
NF = BS // FCH            # chunks (2)

_ACT = mybir.ActivationFunctionType
_ALU = mybir.AluOpType


def _patch_bass():
    """Work around two walrus limitations in this container:
    1. the raw-ISA EVENT_SEMAPHORE_RANGE_CLEAR Tile emits at exit doesn't
       codegen -> skip the clears, keep the sem accounting;
    2. instructions with more than 1 sync wait fail setupSyncWait ->
       split extras onto EventSemaphore carriers (see _split_multiwaits).
    """
    def _cafs(self, sems):
        if not sems:
            return
        sem_nums = [s.num if hasattr(s, "num") else s for s in sems]
        self._state.prepend_free_semaphores(sem_nums)
        for ps in self._tile_sem_poison_stack:
            ps.update(sem_nums)

    bass.Bass.clear_and_free_semaphores = _cafs


def _split_multiwaits(nc):
    for blk in nc.m.functions[0].blocks:
        insts = blk.instructions
        i = 0
        while i < len(insts):
            inst = insts[i]
            si = inst.sync_info
            cap = 1
            if si is not None and len(si.on_wait) > cap:
                waits = list(si.on_wait)
                si.on_wait = waits[:cap]
                extra = waits[cap:]
                k = 0
                while extra:
                    chunk, extra = extra[:1], extra[1:]
                    carrier = mybir.InstEventSemaphore(
                        name=f"{inst.name}-ws{k}", engine=inst.engine
                    )
                    carrier.sync_info = bass_rust.SyncInfo(
                        on_wait=chunk, on_update=[]
                    )
                    insts.insert(i, carrier)
                    i += 1
                    k += 1
            i += 1


def build_module(split=True):
    _patch_bass()
    nc = bass.Bass("TRN2", target_bir_lowering=False, debug=False)

    cT0 = nc.dram_tensor("cT0", [H, BS], F32, kind="ExternalInput").ap()
    hT0 = nc.dram_tensor("hT0", [H, BS], F16, kind="ExternalInput").ap()
    xB = nc.dram_tensor("xB", [128, BS], F16, kind="ExternalInput").ap()
    w0 = nc.dram_tensor("w0", [JT, 128, 4 * H], F16, kind="ExternalInput").ap()
    wl = nc.dram_tensor("wl", [L - 1, JT, 128, 4 * H], F16, kind="ExternalInput").ap()
    wx0 = nc.dram_tensor("wx0", [128, 32], F32, kind="ExternalInput").ap()
    bias = nc.dram_tensor("bias", [128, L * 32], F32, kind="ExternalInput").ap()
    wd = nc.dram_tensor("wd", [128, KT], F16, kind="ExternalInput").ap()
    bd = nc.dram_tensor("bd", [1, 1], F32, kind="ExternalInput").ap()

    cT_out = nc.dram_tensor("cT_out", [H, BS], F32, kind="ExternalOutput").ap()
    hT_out = nc.dram_tensor("hT_out", [H, BS], F32, kind="ExternalOutput").ap()
    xp_out = nc.dram_tensor("xp_out", [1, BS], F32, kind="ExternalOutput").ap()

    with TileContext(nc) as tc:
        with (
            tc.tile_pool(name="persist", bufs=1) as persist,
            tc.tile_pool(name="wpool", bufs=4) as wpool,
            tc.tile_pool(name="psum", bufs=6, space="PSUM") as psum,
            tc.tile_pool(name="dpsum", bufs=2, space="PSUM") as dpsum,
            tc.tile_pool(name="gact", bufs=2) as gact,
            tc.tile_pool(name="gtmp", bufs=2) as gtmp,
            tc.tile_pool(name="hout", bufs=3) as hout,
        ):
            # persistent state
            c_t = [persist.tile([128, BS], F32, tag=f"c{j}", name=f"c{j}") for j in range(JT)]
            h_a = [persist.tile([128, BS], F16, tag=f"ha{j}", name=f"ha{j}") for j in range(JT)]
            h_b = [persist.tile([128, BS], F16, tag=f"hb{j}", name=f"hb{j}") for j in range(JT)]
            bias_sb = persist.tile([128, L * 32], F32, tag="bias")
            x_sb = persist.tile([128, BS], F16, tag="x")
            wx0_sb = persist.tile([128, 32], F32, tag="wx0")
            wd_sb = persist.tile([128, KT], F16, tag="wd")
            bd_sb = persist.tile([1, 1], F32, tag="bd")

            w_tiles = {}

            # HAM warm-up: ~14 scratch matmuls (~5us) bridge the DMA wait so
            # HAM un-throttles before the first real matmul arrives.
            warm = persist.tile([128, 512], F16, tag="warm")
            nc.vector.memset(warm, 0.0)
            warm_ps = dpsum.tile([128, FCH], F32, tag="dp", name="warmps")
            for _ in range(14):
                nc.tensor.matmul(warm_ps, lhsT=warm[:, 0:128], rhs=warm,
                                 start=True, stop=True)

            def load_w(l, j):
                t = wpool.tile([128, 4 * H], F16, tag="w", name=f"w{l}_{j}")
                src = w0[j] if l == 0 else wl[l - 1][j]
                # split across DMA queues: one queue moves only ~50GB/s
                for q in range(4):
                    qs = slice(q * H, (q + 1) * H)
                    nc.sync.dma_start(out=t[:, qs], in_=src[:, qs])
                return t

            def load_c(j):
                for q in range(NF):
                    qs = slice(q * FCH, (q + 1) * FCH)
                    nc.sync.dma_start(out=c_t[j][:, qs],
                                      in_=cT0[j * 128:(j + 1) * 128, qs])

            # PE-critical loads first: weight slab j=0 (HWDGE queues) +
            # h chunk 0 on the SWDGE queues so the burst uses all 16.
            w_tiles[(0, 0)] = load_w(0, 0)
            for j in range(JT):
                nc.gpsimd.dma_start(out=h_a[j][:, 0:FCH],
                                    in_=hT0[j * 128:(j + 1) * 128, 0:FCH])
            nc.sync.dma_start(out=bias_sb, in_=bias[:, :])
            nc.sync.dma_start(out=wx0_sb, in_=wx0[:, :])
            nc.sync.dma_start(out=wd_sb, in_=wd[:, :])
            nc.sync.dma_start(out=bd_sb, in_=bd[:, :])
            load_c(0)
            nc.sync.dma_start(out=x_sb, in_=xB[:, :])
            for j in range(JT):
                nc.sync.dma_start(out=h_a[j][:, FCH:BS],
                                  in_=hT0[j * 128:(j + 1) * 128, FCH:BS])
            w_tiles[(0, 1)] = load_w(0, 1)
            load_c(1)
            w_tiles[(0, 2)] = load_w(0, 2)
            w_tiles[(0, 3)] = load_w(0, 3)
            for j in range(2, JT):
                load_c(j)

            for l in range(L):
                h_in = h_a if l % 2 == 0 else h_b
                h_out = h_b if l % 2 == 0 else h_a
                for j in range(JT):
                    w_sb = w_tiles.pop((l, j), None)
                    if w_sb is None:
                        w_sb = load_w(l, j)
                    for f in range(NF):
                        fsl = slice(f * FCH, (f + 1) * FCH)
                        ps = []
                        for g in range(4):
                            p = psum.tile([128, FCH], F32, tag="ps", name="ps")
                            ps.append(p)
                            for k in range(KT):
                                nc.tensor.matmul(
                                    p,
                                    lhsT=w_sb[:, (k * 4 + g) * 128:(k * 4 + g + 1) * 128],
                                    rhs=h_in[k][:, fsl],
                                    start=(k == 0),
                                    stop=(k == KT - 1),
                                )
                            if l == 0:
                                # += Wx0[n] * x[m]  (rank-1 x-term on DVE)
                                nc.vector.scalar_tensor_tensor(
                                    out=p,
                                    in0=x_sb[:, fsl],
                                    scalar=wx0_sb[:, j * 4 + g:j * 4 + g + 1],
                                    in1=p,
                                    op0=_ALU.mult,
                                    op1=_ALU.add,
                                )
                        bcol = lambda g: bias_sb[:, l * 32 + j * 4 + g:l * 32 + j * 4 + g + 1]
                        i_t = gact.tile([128, FCH], F32, tag="i")
                        nc.scalar.activation(i_t, ps[0], _ACT.Sigmoid, bias=bcol(0))
                        f_t = gact.tile([128, FCH], F32, tag="f")
                        nc.scalar.activation(f_t, ps[1], _ACT.Sigmoid, bias=bcol(1))
                        g_t = gact.tile([128, FCH], F32, tag="g")
                        nc.scalar.activation(g_t, ps[2], _ACT.Tanh, bias=bcol(2))
                        o_t = gact.tile([128, FCH], F32, tag="o")
                        nc.scalar.activation(o_t, ps[3], _ACT.Sigmoid, bias=bcol(3))

                        csl = c_t[j][:, fsl]
                        t1 = gtmp.tile([128, FCH], F32, tag="t1")
                        nc.vector.tensor_mul(t1, f_t, csl)
                        t2 = gtmp.tile([128, FCH], F32, tag="t2")
                        nc.vector.tensor_mul(t2, i_t, g_t)
                        nc.vector.tensor_add(csl, t1, t2)
                        th = gtmp.tile([128, FCH], F32, tag="th")
                        nc.scalar.activation(th, csl, _ACT.Tanh)
                        if l < L - 1:
                            nc.vector.tensor_mul(h_out[j][:, fsl], o_t, th)
                        else:
                            hf = hout.tile([128, FCH], F32, tag="hf")
                            nc.vector.tensor_mul(hf, o_t, th)
                            nc.vector.tensor_copy(h_out[j][:, fsl], hf)
                            nc.sync.dma_start(
                                out=hT_out[j * 128:(j + 1) * 128, fsl], in_=hf
                            )
                            nc.sync.dma_start(
                                out=cT_out[j * 128:(j + 1) * 128, fsl], in_=csl
                            )

            # dense head: x_pred = h4 @ Wd + bd  ->  [1, BS]
            h_fin = h_a if L % 2 == 0 else h_b
            for f in range(NF):
                fsl = slice(f * FCH, (f + 1) * FCH)
                dp = dpsum.tile([1, FCH], F32, tag="dp")
                for k in range(KT):
                    nc.tensor.matmul(
                        dp,
                        lhsT=wd_sb[:, k:k + 1],
                        rhs=h_fin[k][:, fsl],
                        start=(k == 0),
                        stop=(k == KT - 1),
                    )
                xp_sb = hout.tile([1, FCH], F32, tag="xp")
                nc.scalar.activation(xp_sb, dp, _ACT.Identity, bias=bd_sb[0:1, 0:1])
                nc.sync.dma_start(out=xp_out[0:1, fsl], in_=xp_sb)

    if split:
        _split_multiwaits(nc)
    return nc


def preprocess(x_todec, c0, h0, Wx0, Wh0, b0, Wx, Wh, b, Wd, bd):
    """Host-side packing into the per-core DRAM layouts."""
    def pack_w(W):
        # W: [H, 4H] -> [j, p, (k g n')] with value W[k*128+p, g*1024+j*128+n']
        Wr = W.reshape(KT, 128, 4, JT, 128)           # [k, p, g, j, n']
        return np.ascontiguousarray(
            Wr.transpose(3, 1, 0, 2, 4).reshape(JT, 128, 4 * H)
        ).astype(NPF16)

    w0_p = pack_w(Wh0)
    wl_p = np.stack([pack_w(Wx[i] + Wh[i]) for i in range(L - 1)])

    def pack_b(bv):
        # [4H] -> [p, j*4+g] with value bv[g*1024 + j*128 + p]
        return bv.reshape(4, JT, 128).transpose(2, 1, 0).reshape(128, 32)

    bias_p = np.ascontiguousarray(
        np.concatenate([pack_b(b0)] + [pack_b(b[i]) for i in range(L - 1)], axis=1)
    ).astype(np.float32)

    wx0_p = np.ascontiguousarray(pack_b(Wx0[0])).astype(np.float32)
    wd_p = np.ascontiguousarray(Wd.reshape(KT, 128).T).astype(NPF16)
    bd_p = bd.reshape(1, 1).astype(np.float32)

    in_maps = []
    for i in range(NCORES):
        sl = slice(i * BS, (i + 1) * BS)
        xb = np.broadcast_to(
            x_todec[sl].reshape(1, BS).astype(NPF16), (128, BS)
        ).copy()
        in_maps.append({
            "cT0": np.ascontiguousarray(c0[sl].T).astype(np.float32),
            "hT0": np.ascontiguousarray(h0[sl].T).astype(NPF16),
            "xB": xb,
            "w0": w0_p,
            "wl": wl_p,
            "wx0": wx0_p,
            "bias": bias_p,
            "wd": wd_p,
            "bd": bd_p,
        })
    return in_maps


def postprocess(results):
    c = np.empty((B, H), np.float32)
    h = np.empty((B, H), np.float32)
    xp = np.empty((B, 1), np.float32)
    for i in range(NCORES):
        sl = slice(i * BS, (i + 1) * BS)
        c[sl] = results[i]["cT_out"].T
        h[sl] = results[i]["hT_out"].T
        xp[sl, 0] = results[i]["xp_out"][0]
    return c, h, xp


_NC_CACHE = None


def kernel(x_todec, c0, h0, Wx0, Wh0, b0, Wx, Wh, b, Wd, bd, training=0, **_):
    global _NC_CACHE
    args = [np.asarray(a, np.float32) for a in
            (x_todec, c0, h0, Wx0, Wh0, b0, Wx, Wh, b, Wd, bd)]
    in_maps = preprocess(*args)
    if _NC_CACHE is None:
        _NC_CACHE = build_module()
    res = run_bass_kernel_spmd(_NC_CACHE, in_maps, list(range(NCORES)))
    return postprocess(res.results)
